# revision 1
# baseline (speedup 1.0000x reference)
"""Distributed Trainium2 kernel for EnhancedSelfAttention (causal attention
with additive ALiBi |i-j| bias) on 8 NeuronCores.

Math: for queries i and keys j<=i the bias is slope*(i-j), so
softmax_j(S_ij + slope*(i-j)) == softmax_j(S_ij - slope*j) — the slope*i term
is constant per row and cancels. Folding w_j = exp(-slope*j) into V's rows
(plus an appended w column for the denominator) turns the whole softmax into
exp(S) followed by a single PV matmul and a divide. w_j underflows to exactly
0 beyond j ~ 75/slope, so early heads only need the first few key blocks.

Sharding: 8 cores = 2 batches x 4 head groups (4 heads each). Each core
computes its partial projection output; partials are summed on the host.
All cores run one SPMD program: per-slot key-block budgets [16, 16, 10, 3]
with heads assigned to slots so that each head's needed blocks <= budget.

Attention works on S^T tiles ([key, query] layout) so the PV contraction
needs no transposes; exp runs on 1024-wide strips (2 key blocks) to amortize
ACT instruction overhead; the divide broadcasts den (fp16) via a ones-matmul
then applies reciprocal_approx_fast.
"""

import sys
import types

import numpy as np

import concourse.bass as bass
import concourse.mybir as mybir
import concourse.tile as tile
from concourse import bacc
from concourse.bass_utils import run_bass_kernel_spmd


def _ensure_axon_hooks():
    """concourse's trace path imports antenv.axon_hooks, which this image
    lacks; give it a no-op fallback so BASS_TRACE=1 can't crash the run."""
    try:
        import antenv.axon_hooks  # noqa: F401
    except Exception:
        try:
            import antenv
            mod = types.ModuleType("antenv.axon_hooks")
            mod.get_axon_ntff_profile_hook = lambda: None
            mod.set_axon_ntff_profile_hook = lambda h: None
            sys.modules["antenv.axon_hooks"] = mod
            antenv.axon_hooks = mod
        except Exception:
            pass


_ensure_axon_hooks()

F32 = mybir.dt.float32
F16 = mybir.dt.float16
ExpF = mybir.ActivationFunctionType.Exp

B, T, C = 2, 2048, 1024
NH, D = 16, 64
P = 128
NT = T // P            # 16 t tiles
KC = C // P            # 8 contraction subtiles for qkv/proj
QCH = 4                # q chunks of 512
KBUD = (16, 16, 10, 3)  # per-slot key-block budgets
N_CORES = 8

# head -> (group, slot): slot0 gets h8,h10,h12,h14; slot1 h9,h11,h13,h15;
# slot2 h4..h7; slot3 h0..h3.  group g heads:
GROUP_HEADS = [(8 + 2 * g, 9 + 2 * g, 4 + g, g) for g in range(4)]

TRACE = False  # test harness sets kernel.TRACE = True for NTFF profiling

_CACHE = {}


def _slopes():
    i = np.arange(1, NH + 1, dtype=np.float64)
    return (1.0 / np.power(2.0, 8.0 * i / NH)).astype(np.float64)


def _build_program():
    nc = bacc.Bacc("TRN2", target_bir_lowering=False, debug=False,
                   num_devices=N_CORES)

    xt_d = nc.dram_tensor("xt", [C, T], F16, kind="ExternalInput").ap()
    wq_d = nc.dram_tensor("wq", [C, 4 * D], F16, kind="ExternalInput").ap()
    wk_d = nc.dram_tensor("wk", [C, 4 * D], F16, kind="ExternalInput").ap()
    wv_d = nc.dram_tensor("wv", [C, 4 * D], F16, kind="ExternalInput").ap()
    wp_d = nc.dram_tensor("wp", [4 * D, C], F16, kind="ExternalInput").ap()
    wcol_d = nc.dram_tensor("wcol", [T, 4], F32, kind="ExternalInput").ap()
    masks_d = nc.dram_tensor("masks", [P, 4 * 512], F16, kind="ExternalInput").ap()
    y_d = nc.dram_tensor("y", [T, C], F16, kind="ExternalOutput").ap()

    with tile.TileContext(nc) as tc:
        with (
            nc.allow_low_precision(reason="fp16 matmul operands by design"),
            tc.tile_pool(name="const", bufs=1) as const,
            tc.tile_pool(name="psB", bufs=2, space="PSUM") as psB,
            tc.tile_pool(name="psO", bufs=3, space="PSUM") as psO,
            tc.tile_pool(name="psR", bufs=1, space="PSUM") as psR,
            tc.tile_pool(name="pp", bufs=4) as pp,
            tc.tile_pool(name="rr", bufs=3) as rr,
            tc.tile_pool(name="rbp", bufs=3) as rbp,
            tc.tile_pool(name="yp", bufs=4) as yp,
        ):
            # ---- persistent SBUF loads
            # Weights first (small), then xt in nch-major order: a QKV group
            # (m, nch) contracts over all 8 k-chunks but reads only its own
            # 512-column slice, so column-major arrival lets the first group
            # finish after ~2MB instead of the full 4MB.
            wq_sb = const.tile([P, KC * 4 * D], F16, tag="wq")
            wk_sb = const.tile([P, KC * 4 * D], F16, tag="wk")
            wv_sb = const.tile([P, KC * 4 * D], F16, tag="wv")
            for w_sb, w_d in ((wq_sb, wq_d), (wk_sb, wk_d), (wv_sb, wv_d)):
                for k in range(KC):
                    nc.sync.dma_start(w_sb[:, k * 256:(k + 1) * 256],
                                      w_d[k * P:(k + 1) * P, :])
            xt_sb = const.tile([P, KC * T], F16, tag="xt")       # 32KB/part
            # left halves of every k-chunk first: the first QKV groups
            # (nch 0/1) can start after 2MB arrives instead of 4MB
            for h in range(2):
                for k in range(KC):
                    nc.sync.dma_start(
                        xt_sb[:, k * T + h * 1024: k * T + (h + 1) * 1024],
                        xt_d[k * P:(k + 1) * P, h * 1024:(h + 1) * 1024])
            # DMA issue order follows first use: wcol feeds the first V
            # eviction (~27us), masks the first diagonal strip (~65us), wp
            # only the projection (~140us).
            wcol_sb = const.tile([P, NT, 4], F32, tag="wcol")
            nc.sync.dma_start(
                wcol_sb[:], wcol_d.rearrange("(n p) c -> p n c", p=P))
            masks_sb = const.tile([P, 4 * 512], F16, tag="masks")
            nc.sync.dma_start(masks_sb[:], masks_d[:])
            wp_sb = const.tile([P, 2 * C], F16, tag="wp")
            for j in range(2):
                nc.sync.dma_start(wp_sb[:, j * C:(j + 1) * C],
                                  wp_d[j * P:(j + 1) * P, :])
            ones_sb = const.tile([1, D], F16, tag="ones")
            nc.any.memset(ones_sb[:], 1.0)
            # warm the ACT exp table during the DMA wait
            warm_sb = const.tile([1, D], F16, tag="warm")
            nc.scalar.activation(warm_sb[:], ones_sb[:], ExpF)

            qt_sb = [const.tile([P, T], F16, tag=f"qt{m}", name=f"qt{m}")
                     for m in range(2)]
            kt_sb = [const.tile([P, T], F16, tag=f"kt{m}", name=f"kt{m}")
                     for m in range(2)]
            vv_sb = const.tile([P, NT, 4, 65], F16, tag="vv")
            ot_sb = [const.tile([P, T], F16, tag=f"ot{m}", name=f"ot{m}")
                     for m in range(2)]

            # ---- phase 1: QT/KT ([d, t] layout) interleaved with V groups so
            # each group's DVE evictions overlap the other stream's matmuls.
            def emit_qkt_group(i, pool=None):
                w_sb, dst = ((wq_sb, qt_sb), (wk_sb, kt_sb))[i // 8]
                m, nch = divmod(i % 8, QCH)
                if pool is None:
                    ps = psB.tile([P, 1024], F32, tag="mm", name="ps_qkt")
                else:
                    ps = pool.tile([P, 512], F32, tag="rb", name="ps_fill")
                for k in range(KC):
                    nc.tensor.matmul(
                        ps[:, 0:512],
                        w_sb[:, k * 256 + m * P: k * 256 + (m + 1) * P],
                        xt_sb[:, k * T + nch * 512: k * T + (nch + 1) * 512],
                        start=(k == 0), stop=(k == KC - 1))
                nc.vector.tensor_copy(
                    dst[m][:, nch * 512:(nch + 1) * 512], ps[:, 0:512])

            def emit_v_group(mt):
                psv = psB.tile([P, 1024], F32, tag="mm", name="ps_v")
                for k in range(KC):
                    nc.tensor.matmul(
                        psv[:, 0:256],
                        xt_sb[:, k * T + mt * P: k * T + (mt + 1) * P],
                        wv_sb[:, k * 256:(k + 1) * 256],
                        start=(k == 0), stop=(k == KC - 1))
                for s in range(4):
                    nc.vector.tensor_scalar_mul(
                        vv_sb[:, mt, s, 0:D], psv[:, s * D:(s + 1) * D],
                        wcol_sb[:, mt, s: s + 1])

            # den columns for all (t, slot) in one strided copy (independent
            # of the V matmuls — disjoint byte ranges of vv)
            nc.vector.tensor_copy(vv_sb[:, :, :, 64], wcol_sb[:])

            # ---- phase 2: attention, flat software pipeline over strips.
            # Each strip = up to 2 key blocks of S^T for one (slot, q-chunk).
            # Issue order per step: S matmuls(i), exp/mask(i), PV(i-1) — the
            # PE queue is in-order, so PV lags one strip behind its exp.
            # Divides are deferred 2 strips past opsum completion so the
            # broadcast matmul never blocks the PE on a DVE dependency.
            # Interleave V groups with QT/KT groups upfront so each group's
            # DVE evictions overlap the other stream's matmuls.
            # m0 QKT groups + all V upfront; the m1 QKT groups (only needed
            # by slots 2/3, i.e. strip index >= 40) are injected as PE filler
            # into the ACT-bound slots-0/1 attention stream below.
            # KT m1 n3 is never read -> skipped.
            qkt_m0 = [0, 8, 1, 9, 2, 10, 3, 11]
            for i in range(16):
                if i < len(qkt_m0):
                    emit_qkt_group(qkt_m0[i])
                emit_v_group(i)
            fillers = [4, 12, 5, 13, 6, 14, 7]

            strips = []
            for s in range(4):
                for qc in range(QCH):
                    kmax = min(KBUD[s], 4 * qc + 4)
                    for g in range((kmax + 1) // 2):
                        kts = [kt for kt in (2 * g, 2 * g + 1) if kt < kmax]
                        strips.append((s, qc, g, kts, kmax))

            opsums = {}        # (s, qc) -> psum tile
            pending = None     # (strip, pst)
            divides = []       # [(emit_at_index, (s, qc))]

            def emit_pv(strip, pst):
                s, qc, g, kts, kmax = strip
                for d_, kt in enumerate(kts):
                    nc.tensor.matmul(
                        opsums[(s, qc)][:],
                        vv_sb[:, kt, s, :],
                        pst[:, d_ * 512:(d_ + 1) * 512],
                        start=(kt == 0), stop=(kt == kmax - 1))

            def emit_divide(s, qc):
                opsum = opsums.pop((s, qc))
                ot_t = ot_sb[s // 2]
                base = (s % 2) * D
                dh = rr.tile([1, 512], F16, tag="dh", name="dh")
                nc.vector.tensor_copy(dh[:], opsum[64:65, :])
                rb = psR.tile([D, 512], F32, tag="rb", name="rb")
                nc.tensor.matmul(rb[:], ones_sb[:], dh[:],
                                 start=True, stop=True)
                rbs = rbp.tile([D, 512], F32, tag="rbs", name="rbs")
                nc.vector.reciprocal_approx_fast(rbs[:], rb[:])
                nc.vector.tensor_mul(
                    ot_t[base:base + D, qc * 512:(qc + 1) * 512],
                    opsum[0:64, :], rbs[:])

            for i, strip in enumerate(strips):
                s, qc, g, kts, kmax = strip
                if fillers and 4 <= i and i % 5 == 4:
                    emit_qkt_group(fillers.pop(0), pool=psR)
                if (s, qc) not in opsums:
                    opsums[(s, qc)] = psO.tile([65, 512], F32, tag="o",
                                               name="opsum")
                qt_t = qt_sb[s // 2]
                kt_t = kt_sb[s // 2]
                base = (s % 2) * D
                w = len(kts)
                sps = psB.tile([P, 1024], F32, tag="mm", name="sps")
                for d_, kt in enumerate(kts):
                    nc.tensor.matmul(
                        sps[:, d_ * 512:(d_ + 1) * 512],
                        kt_t[base:base + D, kt * P:(kt + 1) * P],
                        qt_t[base:base + D, qc * 512:(qc + 1) * 512],
                        start=True, stop=True)
                pst = pp.tile([P, 1024], F16, tag="p", name="pst")
                nc.scalar.activation(pst[:, 0:512 * w], sps[:, 0:512 * w], ExpF)
                if g == 2 * qc:  # diagonal blocks delta 0,1
                    nc.vector.tensor_mul(pst[:, 0:512 * w], pst[:, 0:512 * w],
                                         masks_sb[:, 0:512 * w])
                elif g == 2 * qc + 1:  # diagonal blocks delta 2,3
                    nc.vector.tensor_mul(pst[:, 0:512 * w], pst[:, 0:512 * w],
                                         masks_sb[:, 1024:1024 + 512 * w])
                while divides and divides[0][0] <= i:
                    emit_divide(*divides.pop(0)[1])
                if pending is not None:
                    emit_pv(*pending)
                    ps_, qc_ = pending[0][0], pending[0][1]
                    if (s, qc) != (ps_, qc_):  # pending was last strip of its
                        divides.append((i + 2, (ps_, qc_)))  # (s,qc): divide
                pending = (strip, pst)
            emit_pv(*pending)
            divides.append((0, (pending[0][0], pending[0][1])))
            # Hoist the first two proj groups' j=0 matmuls (they read only
            # ot_sb[0], complete since slot 1) ahead of the serial divide
            # tail so the PE keeps working through it.
            early_ps = []
            for mt in range(2):
                ps = psB.tile([P, 1024], F32, tag="mm", name="ps_proj_e")
                for nch in range(2):
                    nc.tensor.matmul(
                        ps[:, nch * 512:(nch + 1) * 512],
                        ot_sb[0][:, mt * P:(mt + 1) * P],
                        wp_sb[:, nch * 512:(nch + 1) * 512],
                        start=True, stop=False)
                early_ps.append(ps)
            for _, key in divides:
                emit_divide(*key)

            # ---- phase 3: partial projection y = OT.T @ wp
            # [128,1024] psum per t-tile (4 matmuls); evictions alternate
            # between scalar and vector engines; output DMA split in two.
            for mt in range(NT):
                if mt < 2:
                    ps = early_ps[mt]
                    for nch in range(2):
                        nc.tensor.matmul(
                            ps[:, nch * 512:(nch + 1) * 512],
                            ot_sb[1][:, mt * P:(mt + 1) * P],
                            wp_sb[:, C + nch * 512: C + (nch + 1) * 512],
                            start=False, stop=True)
                else:
                    ps = psB.tile([P, 1024], F32, tag="mm", name="ps_proj")
                    for nch in range(2):
                        for j in range(2):
                            nc.tensor.matmul(
                                ps[:, nch * 512:(nch + 1) * 512],
                                ot_sb[j][:, mt * P:(mt + 1) * P],
                                wp_sb[:, j * C + nch * 512: j * C + (nch + 1) * 512],
                                start=(j == 0), stop=(j == 1))
                yt = yp.tile([P, 1024], F16, tag="y", name="yt")
                if mt % 2 == 0:
                    nc.scalar.copy(yt[:], ps[:])
                else:
                    nc.vector.tensor_copy(yt[:], ps[:])
                for h in range(2):
                    nc.sync.dma_start(
                        y_d[mt * P:(mt + 1) * P, h * 512:(h + 1) * 512],
                        yt[:, h * 512:(h + 1) * 512])

    nc.compile()
    return nc


def _host_prep(x, w_qkv, w_proj):
    """Per-core input maps."""
    slopes = _slopes()
    scale = 1.0 / np.sqrt(D)
    in_maps = []
    xt_by_b = [np.ascontiguousarray(x[b].T).astype(np.float16) for b in range(B)]

    # masks: delta in 0..3, [128, 512] each: valid iff r <= c - 128*delta
    rr_ = np.arange(P)[:, None]
    cc = np.arange(512)[None, :]
    masks = np.concatenate(
        [(rr_ <= cc - P * d).astype(np.float16) for d in range(4)], axis=1)

    group_data = []
    for g in range(4):
        H = GROUP_HEADS[g]
        cols = np.concatenate([np.arange(h * D, (h + 1) * D) for h in H])
        wq = (w_qkv[:, cols] * scale).astype(np.float16)
        wk = w_qkv[:, C + cols].astype(np.float16)
        wv = w_qkv[:, 2 * C + cols].astype(np.float16)
        wp = np.ascontiguousarray(w_proj[cols, :]).astype(np.float16)
        t = np.arange(T, dtype=np.float64)
        wcol = np.stack(
            [np.exp(-slopes[h] * t) for h in H], axis=1).astype(np.float32)
        group_data.append((wq, wk, wv, wp, wcol))

    for c in range(N_CORES):
        b, g = divmod(c, 4)
        wq, wk, wv, wp, wcol = group_data[g]
        in_maps.append({
            "xt": xt_by_b[b], "wq": wq, "wk": wk, "wv": wv, "wp": wp,
            "wcol": wcol, "masks": masks,
        })
    return in_maps


def kernel(x, w_qkv, w_proj):
    if "nc" not in _CACHE:
        _CACHE["nc"] = _build_program()
    nc = _CACHE["nc"]

    in_maps = _host_prep(np.asarray(x, np.float32), np.asarray(w_qkv, np.float32),
                         np.asarray(w_proj, np.float32))
    res = run_bass_kernel_spmd(nc, in_maps, list(range(N_CORES)), trace=TRACE)
    _CACHE["last_result"] = res

    y = np.zeros((B, T, C), dtype=np.float64)
    for c in range(N_CORES):
        b = c // 4
        y[b] += res.results[c]["y"].astype(np.float64)
    return y.astype(np.float32)



# revision 4
# speedup vs baseline: 1.1375x; 1.1375x over previous
"""Distributed Trainium2 kernel for EnhancedSelfAttention (causal attention
with additive ALiBi |i-j| bias) on 8 NeuronCores.

Math: for queries i and keys j<=i the bias is slope*(i-j), so
softmax_j(S_ij + slope*(i-j)) == softmax_j(S_ij - slope*j) — the slope*i term
is constant per row and cancels. Folding w_j = exp(-slope*j) into V's rows
(plus an appended w column for the denominator) turns the whole softmax into
exp(S) followed by a single PV matmul and a divide. w_j decays so fast that
head h only needs keys with slope_h*j < ~24 (beyond that the dropped weight
is < e^-20 of the total): per-head key-block budgets G(h) =
ceil(24/(128*slope_h)) capped at 16.

Sharding: 8 cores = 2 batches x 4 head groups (4 heads each). Heads are
assigned to (group, slot) sorted by budget so the per-slot SPMD budgets
(16, 12, 3, 1) are tight: slot0 heads {15,14,13,12}, slot1 {11,10,9,8},
slot2 {7,6,5,4}, slot3 {3,2,1,0}; group g takes (15-g, 11-g, 7-g, 3-g).
Each core computes its partial projection output; partials summed on host.

Attention works on S^T tiles ([key, query] layout) so the PV contraction
needs no transposes; exp runs on 1024-wide strips (2 key blocks) to amortize
ACT instruction overhead; the divide broadcasts den (fp16) via a ones-matmul
then applies reciprocal_approx_fast.

Input DMAs are consolidated into one large transfer per tensor (small DMAs
are ~650ns latency-bound each; large ones hit ~420GB/s line rate) and split
across both HWDGE queues (sync + scalar) ordered by first use, so the PE
starts at ~7us instead of ~26us.
"""

import sys
import types

import numpy as np

import concourse.bass as bass
import concourse.mybir as mybir
import concourse.tile as tile
from concourse import bacc
from concourse.bass_utils import run_bass_kernel_spmd


def _ensure_axon_hooks():
    """concourse's trace path imports antenv.axon_hooks, which this image
    lacks; give it a no-op fallback so BASS_TRACE=1 can't crash the run."""
    try:
        import antenv.axon_hooks  # noqa: F401
    except Exception:
        try:
            import antenv
            mod = types.ModuleType("antenv.axon_hooks")
            mod.get_axon_ntff_profile_hook = lambda: None
            mod.set_axon_ntff_profile_hook = lambda h: None
            sys.modules["antenv.axon_hooks"] = mod
            antenv.axon_hooks = mod
        except Exception:
            pass


_ensure_axon_hooks()

F32 = mybir.dt.float32
F16 = mybir.dt.float16
ExpF = mybir.ActivationFunctionType.Exp

B, T, C = 2, 2048, 1024
NH, D = 16, 64
P = 128
NT = T // P            # 16 t tiles
KC = C // P            # 8 contraction subtiles for qkv/proj
QCH = 4                # q chunks of 512
KBUD = (16, 12, 3, 1)  # per-slot key-block budgets (see docstring)
N_CORES = 8

# head -> (group, slot): slot s of group g holds head (15-4s... see docstring)
GROUP_HEADS = [(15 - g, 11 - g, 7 - g, 3 - g) for g in range(4)]

# V eviction / matmul width per t-tile: slot s only consumes key blocks
# kt < KBUD[s], so t-tiles beyond a slot's budget skip that slot's V columns.
def _v_slots(mt):
    return [s for s in range(4) if mt < KBUD[s]]


TRACE = False  # test harness sets kernel.TRACE = True for NTFF profiling

_CACHE = {}


def _slopes():
    i = np.arange(1, NH + 1, dtype=np.float64)
    return (1.0 / np.power(2.0, 8.0 * i / NH)).astype(np.float64)


def _build_program():
    nc = bacc.Bacc("TRN2", target_bir_lowering=False, debug=False,
                   num_devices=N_CORES)

    xt_d = nc.dram_tensor("xt", [C, T], F16, kind="ExternalInput").ap()
    wq_d = nc.dram_tensor("wq", [C, 4 * D], F16, kind="ExternalInput").ap()
    wk_d = nc.dram_tensor("wk", [C, 4 * D], F16, kind="ExternalInput").ap()
    wv_d = nc.dram_tensor("wv", [C, 4 * D], F16, kind="ExternalInput").ap()
    wp_d = nc.dram_tensor("wp", [4 * D, C], F16, kind="ExternalInput").ap()
    wcol_d = nc.dram_tensor("wcol", [T, 4], F32, kind="ExternalInput").ap()
    masks_d = nc.dram_tensor("masks", [P, 4 * 512], F16, kind="ExternalInput").ap()
    y_d = nc.dram_tensor("y", [T, C], F16, kind="ExternalOutput").ap()

    with tile.TileContext(nc) as tc:
        with (
            nc.allow_low_precision(reason="fp16 matmul operands by design"),
            tc.tile_pool(name="const", bufs=1) as const,
            tc.tile_pool(name="psB", bufs=2, space="PSUM") as psB,
            tc.tile_pool(name="psO", bufs=3, space="PSUM") as psO,
            tc.tile_pool(name="psR", bufs=1, space="PSUM") as psR,
            tc.tile_pool(name="pp", bufs=4) as pp,
            tc.tile_pool(name="rr", bufs=3) as rr,
            tc.tile_pool(name="rbp", bufs=3) as rbp,
            tc.tile_pool(name="yp", bufs=4) as yp,
        ):
            # ---- persistent SBUF tiles
            wq_sb = const.tile([P, KC * 4 * D], F16, tag="wq")
            wk_sb = const.tile([P, KC * 4 * D], F16, tag="wk")
            wv_sb = const.tile([P, KC * 4 * D], F16, tag="wv")
            xt_sb = const.tile([P, KC * T], F16, tag="xt")       # 32KB/part
            wcol_sb = const.tile([P, NT, 4], F32, tag="wcol")
            masks_sb = const.tile([P, 4 * 512], F16, tag="masks")
            wp_sb = const.tile([P, 2 * C], F16, tag="wp")

            # ---- input DMAs: one large transfer each (line rate), split
            # across the two HWDGE queues (sync + scalar/Activation), ordered
            # by first use. First matmul needs wq + xt nch0.
            def xt_nch_dma(eng, n):
                eng.dma_start(
                    xt_sb[:].rearrange("p (k t) -> p k t", k=KC)[
                        :, :, n * 512:(n + 1) * 512],
                    xt_d[:, n * 512:(n + 1) * 512].rearrange(
                        "(k p) t -> p k t", p=P))

            nc.sync.dma_start(
                wq_sb[:].rearrange("p (k c) -> p k c", k=KC),
                wq_d.rearrange("(k p) c -> p k c", p=P))
            xt_nch_dma(nc.sync, 0)
            xt_nch_dma(nc.sync, 1)
            nc.sync.dma_start(masks_sb[:], masks_d[:])
            nc.sync.dma_start(
                wp_sb[:].rearrange("p (j c) -> p j c", j=2),
                wp_d.rearrange("(j p) c -> p j c", p=P))

            nc.scalar.dma_start(
                wk_sb[:].rearrange("p (k c) -> p k c", k=KC),
                wk_d.rearrange("(k p) c -> p k c", p=P))
            nc.scalar.dma_start(
                wv_sb[:].rearrange("p (k c) -> p k c", k=KC),
                wv_d.rearrange("(k p) c -> p k c", p=P))
            nc.scalar.dma_start(
                wcol_sb[:], wcol_d.rearrange("(n p) c -> p n c", p=P))
            xt_nch_dma(nc.scalar, 2)
            xt_nch_dma(nc.scalar, 3)

            ones_sb = const.tile([1, D], F16, tag="ones")
            nc.any.memset(ones_sb[:], 1.0)
            # warm the ACT exp table during the DMA wait
            warm_sb = const.tile([1, D], F16, tag="warm")
            nc.scalar.activation(warm_sb[:], ones_sb[:], ExpF)

            qt_sb = [const.tile([P, T], F16, tag=f"qt{m}", name=f"qt{m}")
                     for m in range(2)]
            kt_sb = [const.tile([P, T], F16, tag=f"kt{m}", name=f"kt{m}")
                     for m in range(2)]
            vv_sb = const.tile([P, NT, 4, 65], F16, tag="vv")
            ot_sb = [const.tile([P, T], F16, tag=f"ot{m}", name=f"ot{m}")
                     for m in range(2)]

            # ---- phase 1: QT/KT ([d, t] layout) + weighted V, interleaved
            # so each group's DVE evictions overlap the other stream's
            # matmuls.  i encodes (w, m, nch): i//8: 0=wq 1=wk;
            # (i%8)//4 = m; i%4 = nch.
            def emit_qkt_group(i):
                w_sb, dst = ((wq_sb, qt_sb), (wk_sb, kt_sb))[i // 8]
                m, nch = divmod(i % 8, QCH)
                ps = psB.tile([P, 1024], F32, tag="mm", name="ps_qkt")
                for k in range(KC):
                    nc.tensor.matmul(
                        ps[:, 0:512],
                        w_sb[:, k * 256 + m * P: k * 256 + (m + 1) * P],
                        xt_sb[:, k * T + nch * 512: k * T + (nch + 1) * 512],
                        start=(k == 0), stop=(k == KC - 1))
                nc.vector.tensor_copy(
                    dst[m][:, nch * 512:(nch + 1) * 512], ps[:, 0:512])

            def emit_v_group(mt):
                slots = _v_slots(mt)
                nw = slots[-1] * D + D  # used columns are a prefix
                psv = psB.tile([P, 1024], F32, tag="mm", name="ps_v")
                for k in range(KC):
                    nc.tensor.matmul(
                        psv[:, 0:nw],
                        xt_sb[:, k * T + mt * P: k * T + (mt + 1) * P],
                        wv_sb[:, k * 256:k * 256 + nw],
                        start=(k == 0), stop=(k == KC - 1))
                for s in slots:
                    nc.vector.tensor_scalar_mul(
                        vv_sb[:, mt, s, 0:D], psv[:, s * D:(s + 1) * D],
                        wcol_sb[:, mt, s: s + 1])

            # den columns for all (t, slot) in one strided copy (independent
            # of the V matmuls — disjoint byte ranges of vv)
            nc.vector.tensor_copy(vv_sb[:, :, :, 64], wcol_sb[:])

            # KT m1 only needs key columns < 512 (slot2 budget 3 blocks);
            # emission ordered by xt nch tranche so matmuls chase the DMAs.
            qkt_by_tranche = [[0, 8, 4, 12], [1, 9, 5], [2, 10, 6], [3, 11, 7]]
            for n in range(4):
                groups = qkt_by_tranche[n]
                vts = list(range(4 * n, 4 * n + 4))
                for j in range(max(len(groups), len(vts))):
                    if j < len(groups):
                        emit_qkt_group(groups[j])
                    if j < len(vts):
                        emit_v_group(vts[j])

            # ---- phase 2: attention, flat software pipeline over strips.
            # Each strip = up to 2 key blocks of S^T for one (slot, q-chunk).
            # Issue order per step: S matmuls(i), exp/mask(i), PV(i-1) — the
            # PE queue is in-order, so PV lags one strip behind its exp.
            # Divides are deferred 2 strips past opsum completion so the
            # broadcast matmul never blocks the PE on a DVE dependency.
            strips = []
            for s in range(4):
                for qc in range(QCH):
                    kmax = min(KBUD[s], 4 * qc + 4)
                    for g in range((kmax + 1) // 2):
                        kts = [kt for kt in (2 * g, 2 * g + 1) if kt < kmax]
                        strips.append((s, qc, g, kts, kmax))

            opsums = {}        # (s, qc) -> psum tile
            pending = None     # (strip, pst)
            divides = []       # [(emit_at_index, (s, qc))]

            def emit_pv(strip, pst):
                s, qc, g, kts, kmax = strip
                for d_, kt in enumerate(kts):
                    nc.tensor.matmul(
                        opsums[(s, qc)][:],
                        vv_sb[:, kt, s, :],
                        pst[:, d_ * 512:(d_ + 1) * 512],
                        start=(kt == 0), stop=(kt == kmax - 1))

            def emit_divide(s, qc):
                opsum = opsums.pop((s, qc))
                ot_t = ot_sb[s // 2]
                base = (s % 2) * D
                dh = rr.tile([1, 512], F16, tag="dh", name="dh")
                nc.vector.tensor_copy(dh[:], opsum[64:65, :])
                rb = psR.tile([D, 512], F32, tag="rb", name="rb")
                nc.tensor.matmul(rb[:], ones_sb[:], dh[:],
                                 start=True, stop=True)
                rbs = rbp.tile([D, 512], F32, tag="rbs", name="rbs")
                nc.vector.reciprocal_approx_fast(rbs[:], rb[:])
                nc.vector.tensor_mul(
                    ot_t[base:base + D, qc * 512:(qc + 1) * 512],
                    opsum[0:64, :], rbs[:])

            for i, strip in enumerate(strips):
                s, qc, g, kts, kmax = strip
                if (s, qc) not in opsums:
                    opsums[(s, qc)] = psO.tile([65, 512], F32, tag="o",
                                               name="opsum")
                qt_t = qt_sb[s // 2]
                kt_t = kt_sb[s // 2]
                base = (s % 2) * D
                w = len(kts)
                sps = psB.tile([P, 1024], F32, tag="mm", name="sps")
                for d_, kt in enumerate(kts):
                    nc.tensor.matmul(
                        sps[:, d_ * 512:(d_ + 1) * 512],
                        kt_t[base:base + D, kt * P:(kt + 1) * P],
                        qt_t[base:base + D, qc * 512:(qc + 1) * 512],
                        start=True, stop=True)
                pst = pp.tile([P, 1024], F16, tag="p", name="pst")
                nc.scalar.activation(pst[:, 0:512 * w], sps[:, 0:512 * w], ExpF)
                if g == 2 * qc:  # diagonal blocks delta 0,1
                    nc.vector.tensor_mul(pst[:, 0:512 * w], pst[:, 0:512 * w],
                                         masks_sb[:, 0:512 * w])
                elif g == 2 * qc + 1:  # diagonal blocks delta 2,3
                    nc.vector.tensor_mul(pst[:, 0:512 * w], pst[:, 0:512 * w],
                                         masks_sb[:, 1024:1024 + 512 * w])
                while divides and divides[0][0] <= i:
                    emit_divide(*divides.pop(0)[1])
                if pending is not None:
                    emit_pv(*pending)
                    ps_, qc_ = pending[0][0], pending[0][1]
                    if (s, qc) != (ps_, qc_):  # pending was last strip of its
                        divides.append((i + 2, (ps_, qc_)))  # (s,qc): divide
                pending = (strip, pst)
            emit_pv(*pending)
            divides.append((0, (pending[0][0], pending[0][1])))
            # Hoist the first two proj groups' j=0 matmuls (they read only
            # ot_sb[0], complete since slot 1) ahead of the serial divide
            # tail so the PE keeps working through it.
            early_ps = []
            for mt in range(2):
                ps = psB.tile([P, 1024], F32, tag="mm", name="ps_proj_e")
                for nch in range(2):
                    nc.tensor.matmul(
                        ps[:, nch * 512:(nch + 1) * 512],
                        ot_sb[0][:, mt * P:(mt + 1) * P],
                        wp_sb[:, nch * 512:(nch + 1) * 512],
                        start=True, stop=False)
                early_ps.append(ps)
            for _, key in divides:
                emit_divide(*key)

            # ---- phase 3: partial projection y = OT.T @ wp
            # [128,1024] psum per t-tile (4 matmuls); evictions alternate
            # between scalar and vector engines; output DMA alternates queues.
            for mt in range(NT):
                if mt < 2:
                    ps = early_ps[mt]
                    for nch in range(2):
                        nc.tensor.matmul(
                            ps[:, nch * 512:(nch + 1) * 512],
                            ot_sb[1][:, mt * P:(mt + 1) * P],
                            wp_sb[:, C + nch * 512: C + (nch + 1) * 512],
                            start=False, stop=True)
                else:
                    ps = psB.tile([P, 1024], F32, tag="mm", name="ps_proj")
                    for nch in range(2):
                        for j in range(2):
                            nc.tensor.matmul(
                                ps[:, nch * 512:(nch + 1) * 512],
                                ot_sb[j][:, mt * P:(mt + 1) * P],
                                wp_sb[:, j * C + nch * 512: j * C + (nch + 1) * 512],
                                start=(j == 0), stop=(j == 1))
                yt = yp.tile([P, 1024], F16, tag="y", name="yt")
                if mt % 2 == 0:
                    nc.scalar.copy(yt[:], ps[:])
                else:
                    nc.vector.tensor_copy(yt[:], ps[:])
                nc.sync.dma_start(y_d[mt * P:(mt + 1) * P, :], yt[:])

    nc.compile()
    return nc


def _host_prep(x, w_qkv, w_proj):
    """Per-core input maps."""
    slopes = _slopes()
    scale = 1.0 / np.sqrt(D)
    in_maps = []
    xt_by_b = [np.ascontiguousarray(x[b].T).astype(np.float16) for b in range(B)]

    # masks: delta in 0..3, [128, 512] each: valid iff r <= c - 128*delta
    rr_ = np.arange(P)[:, None]
    cc = np.arange(512)[None, :]
    masks = np.concatenate(
        [(rr_ <= cc - P * d).astype(np.float16) for d in range(4)], axis=1)

    group_data = []
    for g in range(4):
        H = GROUP_HEADS[g]
        cols = np.concatenate([np.arange(h * D, (h + 1) * D) for h in H])
        wq = (w_qkv[:, cols] * scale).astype(np.float16)
        wk = w_qkv[:, C + cols].astype(np.float16)
        wv = w_qkv[:, 2 * C + cols].astype(np.float16)
        wp = np.ascontiguousarray(w_proj[cols, :]).astype(np.float16)
        t = np.arange(T, dtype=np.float64)
        wcol = np.stack(
            [np.exp(-slopes[h] * t) for h in H], axis=1).astype(np.float32)
        group_data.append((wq, wk, wv, wp, wcol))

    for c in range(N_CORES):
        b, g = divmod(c, 4)
        wq, wk, wv, wp, wcol = group_data[g]
        in_maps.append({
            "xt": xt_by_b[b], "wq": wq, "wk": wk, "wv": wv, "wp": wp,
            "wcol": wcol, "masks": masks,
        })
    return in_maps


def kernel(x, w_qkv, w_proj):
    if "nc" not in _CACHE:
        _CACHE["nc"] = _build_program()
    nc = _CACHE["nc"]

    in_maps = _host_prep(np.asarray(x, np.float32), np.asarray(w_qkv, np.float32),
                         np.asarray(w_proj, np.float32))
    res = run_bass_kernel_spmd(nc, in_maps, list(range(N_CORES)), trace=TRACE)
    _CACHE["last_result"] = res

    y = np.zeros((B, T, C), dtype=np.float64)
    for c in range(N_CORES):
        b = c // 4
        y[b] += res.results[c]["y"].astype(np.float64)
    return y.astype(np.float32)


# revision 5
# speedup vs baseline: 1.2377x; 1.0881x over previous
"""Distributed Trainium2 kernel for EnhancedSelfAttention (causal attention
with additive ALiBi |i-j| bias) on 8 NeuronCores.

Math: for queries i and keys j<=i the bias is slope*(i-j), so
softmax_j(S_ij + slope*(i-j)) == softmax_j(S_ij - slope*j) — the slope*i term
is constant per row and cancels. Folding w_j = exp(-slope*j) into V's rows
(plus an appended w column for the denominator) turns the whole softmax into
exp(S) followed by a single PV matmul and a divide. w_j decays so fast that
head h only needs keys with slope_h*j < ~24 (beyond that the dropped weight
is < e^-20 of the total).

Sharding: 8 cores = 2 batches x 4 head groups (4 heads each). Heads are
assigned to (group, slot) sorted by budget so the per-slot SPMD budgets
(16, 12, 3, 1) are tight: group g takes heads (15-g, 11-g, 7-g, 3-g) in
slots (0, 1, 2, 3). Each core computes its partial projection output;
partials are summed on the host.

Attention works on S^T tiles ([key, query] layout) so the PV contraction
needs no transposes. Slots are processed in PAIRS (0,1) and (2,3): the pair
shares one [128,1024] PSUM tile — slot a's S block in columns 0:512 via PE
row-tile T0 (SBUF partitions 0:63), slot b's in 512:1024 via T8 (64:127) —
so the two K=64 matmuls run CONCURRENTLY in the 64x128-tiled PE array, and
one 1024-wide exp covers both. The divide broadcasts den via a ones-matmul
(rb tile borrowed from the S psum ring) then reciprocal_approx_fast.

All inputs are pre-swizzled on the host into the exact SBUF layout so every
DMA is a contiguous [128, N] copy at line rate (~420GB/s), split across the
two HWDGE queues (sync + scalar) ordered by first use.
"""

import sys
import types

import numpy as np

import concourse.bass as bass
import concourse.mybir as mybir
import concourse.tile as tile
from concourse import bacc
from concourse.bass_utils import run_bass_kernel_spmd


def _ensure_axon_hooks():
    """concourse's trace path imports antenv.axon_hooks, which this image
    lacks; give it a no-op fallback so BASS_TRACE=1 can't crash the run."""
    try:
        import antenv.axon_hooks  # noqa: F401
    except Exception:
        try:
            import antenv
            mod = types.ModuleType("antenv.axon_hooks")
            mod.get_axon_ntff_profile_hook = lambda: None
            mod.set_axon_ntff_profile_hook = lambda h: None
            sys.modules["antenv.axon_hooks"] = mod
            antenv.axon_hooks = mod
        except Exception:
            pass


_ensure_axon_hooks()

F32 = mybir.dt.float32
F16 = mybir.dt.float16
ExpF = mybir.ActivationFunctionType.Exp

B, T, C = 2, 2048, 1024
NH, D = 16, 64
P = 128
NT = T // P            # 16 t tiles
KC = C // P            # 8 contraction subtiles for qkv/proj
QCH = 4                # q chunks of 512
KBUD = (16, 12, 3, 1)  # per-slot key-block budgets (see docstring)
N_CORES = 8

GROUP_HEADS = [(15 - g, 11 - g, 7 - g, 3 - g) for g in range(4)]

TRACE = False  # test harness sets kernel.TRACE = True for NTFF profiling

_CACHE = {}


def _slopes():
    i = np.arange(1, NH + 1, dtype=np.float64)
    return (1.0 / np.power(2.0, 8.0 * i / NH)).astype(np.float64)


def _build_program():
    nc = bacc.Bacc("TRN2", target_bir_lowering=False, debug=False,
                   num_devices=N_CORES)

    # All host-side arrays are pre-swizzled to [128, free] SBUF layout.
    xt_d = nc.dram_tensor("xt", [P, QCH * KC * 512], F16, kind="ExternalInput").ap()
    wq_d = nc.dram_tensor("wq", [P, KC * 256], F16, kind="ExternalInput").ap()
    wk_d = nc.dram_tensor("wk", [P, KC * 256], F16, kind="ExternalInput").ap()
    wv_d = nc.dram_tensor("wv", [P, KC * 256], F16, kind="ExternalInput").ap()
    wp_d = nc.dram_tensor("wp", [P, 2 * C], F16, kind="ExternalInput").ap()
    wcol_d = nc.dram_tensor("wcol", [P, NT * 4], F32, kind="ExternalInput").ap()
    masks_d = nc.dram_tensor("masks", [P, 4 * 1024], F16, kind="ExternalInput").ap()
    y_d = nc.dram_tensor("y", [T, C], F16, kind="ExternalOutput").ap()

    with tile.TileContext(nc) as tc:
        with (
            nc.allow_low_precision(reason="fp16 matmul operands by design"),
            tc.tile_pool(name="const", bufs=1) as const,
            tc.tile_pool(name="psB", bufs=2, space="PSUM") as psB,
            tc.tile_pool(name="psO", bufs=4, space="PSUM") as psO,
            tc.tile_pool(name="pp", bufs=4) as pp,
            tc.tile_pool(name="rr", bufs=3) as rr,
            tc.tile_pool(name="rbp", bufs=3) as rbp,
            tc.tile_pool(name="yp", bufs=4) as yp,
        ):
            # ---- persistent SBUF tiles
            wq_sb = const.tile([P, KC * 256], F16, tag="wq")
            wk_sb = const.tile([P, KC * 256], F16, tag="wk")
            wv_sb = const.tile([P, KC * 256], F16, tag="wv")
            xt_sb = const.tile([P, QCH * KC * 512], F16, tag="xt")  # 32KB/part
            wcol_sb = const.tile([P, NT, 4], F32, tag="wcol")
            masks_sb = const.tile([P, 4 * 1024], F16, tag="masks")
            wp_sb = const.tile([P, 2 * C], F16, tag="wp")

            # ---- input DMAs: contiguous [128, N] copies at line rate,
            # split across the two HWDGE queues, ordered by first use.
            NX = KC * 512  # xt columns per q-chunk tranche
            nc.sync.dma_start(wq_sb[:], wq_d[:])
            nc.sync.dma_start(xt_sb[:, 0:NX], xt_d[:, 0:NX])
            nc.sync.dma_start(xt_sb[:, NX:2 * NX], xt_d[:, NX:2 * NX])
            nc.sync.dma_start(masks_sb[:], masks_d[:])
            nc.sync.dma_start(wp_sb[:], wp_d[:])

            nc.scalar.dma_start(wk_sb[:], wk_d[:])
            nc.scalar.dma_start(wv_sb[:], wv_d[:])
            nc.scalar.dma_start(
                wcol_sb[:], wcol_d[:].rearrange("p (n c) -> p n c", c=4))
            nc.scalar.dma_start(xt_sb[:, 2 * NX:3 * NX], xt_d[:, 2 * NX:3 * NX])
            nc.scalar.dma_start(xt_sb[:, 3 * NX:4 * NX], xt_d[:, 3 * NX:4 * NX])

            ones_sb = const.tile([1, D], F16, tag="ones")
            nc.any.memset(ones_sb[:], 1.0)
            # warm the ACT exp table during the DMA wait
            warm_sb = const.tile([1, D], F16, tag="warm")
            nc.scalar.activation(warm_sb[:], ones_sb[:], ExpF)

            qt_sb = [const.tile([P, T], F16, tag=f"qt{m}", name=f"qt{m}")
                     for m in range(2)]
            kt_sb = [const.tile([P, T], F16, tag=f"kt{m}", name=f"kt{m}")
                     for m in range(2)]
            vv_sb = const.tile([P, NT, 4, 65], F16, tag="vv")
            ot_sb = [const.tile([P, T], F16, tag=f"ot{m}", name=f"ot{m}")
                     for m in range(2)]

            def xt_ap(nch, k, col, width):
                off = nch * NX + k * 512 + col
                return xt_sb[:, off:off + width]

            # ---- phase 1: QT/KT ([d, t] layout) + weighted V, interleaved.
            # i encodes (w, m, nch): i//8: 0=wq 1=wk; (i%8)//4 = m; i%4 = nch.
            def emit_qkt_group(i):
                w_sb, dst = ((wq_sb, qt_sb), (wk_sb, kt_sb))[i // 8]
                m, nch = divmod(i % 8, QCH)
                ps = psB.tile([P, 1024], F32, tag="mm", name="ps_qkt")
                for k in range(KC):
                    nc.tensor.matmul(
                        ps[:, 0:512],
                        w_sb[:, k * 256 + m * P: k * 256 + (m + 1) * P],
                        xt_ap(nch, k, 0, 512),
                        start=(k == 0), stop=(k == KC - 1))
                nc.vector.tensor_copy(
                    dst[m][:, nch * 512:(nch + 1) * 512], ps[:, 0:512])

            def emit_v_group(mt):
                slots = [s for s in range(4) if mt < KBUD[s]]
                nw = slots[-1] * D + D  # used columns are a prefix
                psv = psB.tile([P, 1024], F32, tag="mm", name="ps_v")
                for k in range(KC):
                    nc.tensor.matmul(
                        psv[:, 0:nw],
                        xt_ap(mt // 4, k, (mt % 4) * P, P),
                        wv_sb[:, k * 256:k * 256 + nw],
                        start=(k == 0), stop=(k == KC - 1))
                for s in slots:
                    nc.vector.tensor_scalar_mul(
                        vv_sb[:, mt, s, 0:D], psv[:, s * D:(s + 1) * D],
                        wcol_sb[:, mt, s: s + 1])

            # den columns for all (t, slot) in one strided copy
            nc.vector.tensor_copy(vv_sb[:, :, :, 64], wcol_sb[:])

            # KT m1 only needs key columns < 512 (slot2 budget is 3 blocks);
            # emission ordered by xt nch tranche so matmuls chase the DMAs.
            qkt_by_tranche = [[0, 8, 4, 12], [1, 9, 5], [2, 10, 6], [3, 11, 7]]
            for n in range(4):
                groups = qkt_by_tranche[n]
                vts = list(range(4 * n, 4 * n + 4))
                for j in range(max(len(groups), len(vts))):
                    if j < len(groups):
                        emit_qkt_group(groups[j])
                    if j < len(vts):
                        emit_v_group(vts[j])

            # ---- phase 2: attention over slot-pair items.
            # item = (a, b, qc, g): key block g of S^T for slots a (cols
            # 0:512, PE row-tile T0) and b (cols 512:1024, T8) — concurrent.
            # Issue order per item: S pair(i), exp/mask(i), PV pair(i-1);
            # divides deferred 2 items past opsum completion.
            items = []
            for (a, b) in ((0, 1), (2, 3)):
                for qc in range(QCH):
                    ka = min(KBUD[a], 4 * qc + 4)
                    kb = min(KBUD[b], 4 * qc + 4)
                    for g in range(max(ka, kb)):
                        items.append((a, b, qc, g, g < ka, g < kb, ka, kb))

            opsums = {}        # (s, qc) -> psum tile
            pending = None     # (item, pst)
            divides = []       # [(emit_at_index, (s, qc))]
            ndiv = [0]

            def emit_pv(item, pst):
                a, b, qc, g, has_a, has_b, ka, kb = item
                if has_a:
                    nc.tensor.matmul(
                        opsums[(a, qc)][:], vv_sb[:, g, a, :], pst[:, 0:512],
                        start=(g == 0), stop=(g == ka - 1))
                if has_b:
                    nc.tensor.matmul(
                        opsums[(b, qc)][:], vv_sb[:, g, b, :],
                        pst[:, 512:1024], start=(g == 0), stop=(g == kb - 1))

            def emit_divide(s, qc):
                opsum = opsums.pop((s, qc))
                ot_t = ot_sb[s // 2]
                base = (s % 2) * D
                dh = rr.tile([1, 512], F16, tag="dh", name="dh")
                if ndiv[0] % 2 == 0:
                    nc.scalar.copy(dh[:], opsum[64:65, :])
                else:
                    nc.vector.tensor_copy(dh[:], opsum[64:65, :])
                ndiv[0] += 1
                rb = psB.tile([P, 1024], F32, tag="mm", name="rb")
                nc.tensor.matmul(rb[0:D, 0:512], ones_sb[:], dh[:],
                                 start=True, stop=True)
                rbs = rbp.tile([D, 512], F32, tag="rbs", name="rbs")
                nc.vector.reciprocal_approx_fast(rbs[:], rb[0:D, 0:512])
                nc.vector.tensor_mul(
                    ot_t[base:base + D, qc * 512:(qc + 1) * 512],
                    opsum[0:64, :], rbs[:])

            for i, item in enumerate(items):
                a, b, qc, g, has_a, has_b, ka, kb = item
                if g == 0:
                    opsums[(a, qc)] = psO.tile([65, 512], F32, tag="o",
                                               name="opsum_a")
                    opsums[(b, qc)] = psO.tile([65, 512], F32, tag="o",
                                               name="opsum_b")
                m = a // 2
                qt_t, kt_t = qt_sb[m], kt_sb[m]
                sps = psB.tile([P, 1024], F32, tag="mm", name="sps")
                if has_a:
                    nc.tensor.matmul(
                        sps[:, 0:512],
                        kt_t[0:D, g * P:(g + 1) * P],
                        qt_t[0:D, qc * 512:(qc + 1) * 512],
                        start=True, stop=True)
                if has_b:
                    nc.tensor.matmul(
                        sps[:, 512:1024],
                        kt_t[D:2 * D, g * P:(g + 1) * P],
                        qt_t[D:2 * D, qc * 512:(qc + 1) * 512],
                        start=True, stop=True)
                pst = pp.tile([P, 1024], F16, tag="p", name="pst")
                lo, hi = (0, 1024) if (has_a and has_b) else (
                    (0, 512) if has_a else (512, 1024))
                nc.scalar.activation(pst[:, lo:hi], sps[:, lo:hi], ExpF)
                delta = g - 4 * qc
                if 0 <= delta <= 3:  # diagonal block: causal mask
                    nc.vector.tensor_mul(
                        pst[:, lo:hi], pst[:, lo:hi],
                        masks_sb[:, delta * 1024: delta * 1024 + (hi - lo)])
                while divides and divides[0][0] <= i:
                    emit_divide(*divides.pop(0)[1])
                if pending is not None:
                    emit_pv(*pending)
                    pa, pb, pqc, pg, pha, phb, pka, pkb = pending[0]
                    if pha and pg == pka - 1:
                        divides.append((i + 2, (pa, pqc)))
                    if phb and pg == pkb - 1:
                        divides.append((i + 2, (pb, pqc)))
                pending = (item, pst)
            emit_pv(*pending)
            pa, pb, pqc, pg, pha, phb, pka, pkb = pending[0]
            if pha and pg == pka - 1:
                divides.append((0, (pa, pqc)))
            if phb and pg == pkb - 1:
                divides.append((0, (pb, pqc)))
            # Hoist the first two proj groups' j=0 matmuls (they read only
            # ot_sb[0], complete since the s01 section) ahead of the divide
            # tail so the PE keeps working through it.
            early_ps = []
            for mt in range(2):
                ps = psB.tile([P, 1024], F32, tag="mm", name="ps_proj_e")
                for nch in range(2):
                    nc.tensor.matmul(
                        ps[:, nch * 512:(nch + 1) * 512],
                        ot_sb[0][:, mt * P:(mt + 1) * P],
                        wp_sb[:, nch * 512:(nch + 1) * 512],
                        start=True, stop=False)
                early_ps.append(ps)
            for _, key in divides:
                emit_divide(*key)

            # ---- phase 3: partial projection y = OT.T @ wp
            for mt in range(NT):
                if mt < 2:
                    ps = early_ps[mt]
                    for nch in range(2):
                        nc.tensor.matmul(
                            ps[:, nch * 512:(nch + 1) * 512],
                            ot_sb[1][:, mt * P:(mt + 1) * P],
                            wp_sb[:, C + nch * 512: C + (nch + 1) * 512],
                            start=False, stop=True)
                else:
                    ps = psB.tile([P, 1024], F32, tag="mm", name="ps_proj")
                    for nch in range(2):
                        for j in range(2):
                            nc.tensor.matmul(
                                ps[:, nch * 512:(nch + 1) * 512],
                                ot_sb[j][:, mt * P:(mt + 1) * P],
                                wp_sb[:, j * C + nch * 512: j * C + (nch + 1) * 512],
                                start=(j == 0), stop=(j == 1))
                yt = yp.tile([P, 1024], F16, tag="y", name="yt")
                if mt % 2 == 0:
                    nc.scalar.copy(yt[:], ps[:])
                else:
                    nc.vector.tensor_copy(yt[:], ps[:])
                nc.sync.dma_start(y_d[mt * P:(mt + 1) * P, :], yt[:])

    nc.compile()
    return nc


def _host_prep(x, w_qkv, w_proj):
    """Per-core input maps, pre-swizzled to SBUF layout [128, free]."""
    slopes = _slopes()
    scale = 1.0 / np.sqrt(D)
    in_maps = []

    # xt: [P, nch, k, 512] with xt[p, n, k, t'] = x[b][n*512+t', k*128+p]
    xt_by_b = []
    for b in range(B):
        xb = x[b].astype(np.float16)  # [T, C]
        sw = np.ascontiguousarray(
            xb.reshape(QCH, 512, KC, P).transpose(3, 0, 2, 1)
        ).reshape(P, QCH * KC * 512)
        xt_by_b.append(sw)

    # masks: delta in 0..3, [128, 1024] each = dup'd [128,512] causal block
    rr_ = np.arange(P)[:, None]
    cc = np.arange(512)[None, :]
    masks = np.concatenate(
        [np.tile((rr_ <= cc - P * d).astype(np.float16), (1, 2))
         for d in range(4)], axis=1)  # [P, 4*1024]

    def swz_w(w):  # [(k p), c] -> [p, (k c)]
        kc = w.shape[1]
        return np.ascontiguousarray(
            w.reshape(KC, P, kc).transpose(1, 0, 2)).reshape(P, KC * kc)

    group_data = []
    for g in range(4):
        H = GROUP_HEADS[g]
        cols = np.concatenate([np.arange(h * D, (h + 1) * D) for h in H])
        wq = swz_w((w_qkv[:, cols] * scale).astype(np.float16))
        wk = swz_w(w_qkv[:, C + cols].astype(np.float16))
        wv = swz_w(w_qkv[:, 2 * C + cols].astype(np.float16))
        wp = np.ascontiguousarray(
            w_proj[cols, :].astype(np.float16).reshape(2, P, C).transpose(1, 0, 2)
        ).reshape(P, 2 * C)
        t = np.arange(T, dtype=np.float64)
        wcol = np.stack(
            [np.exp(-slopes[h] * t) for h in H], axis=1).astype(np.float32)
        wcol = np.ascontiguousarray(
            wcol.reshape(NT, P, 4).transpose(1, 0, 2)).reshape(P, NT * 4)
        group_data.append((wq, wk, wv, wp, wcol))

    for c in range(N_CORES):
        b, g = divmod(c, 4)
        wq, wk, wv, wp, wcol = group_data[g]
        in_maps.append({
            "xt": xt_by_b[b], "wq": wq, "wk": wk, "wv": wv, "wp": wp,
            "wcol": wcol, "masks": masks,
        })
    return in_maps


def kernel(x, w_qkv, w_proj):
    if "nc" not in _CACHE:
        _CACHE["nc"] = _build_program()
    nc = _CACHE["nc"]

    in_maps = _host_prep(np.asarray(x, np.float32), np.asarray(w_qkv, np.float32),
                         np.asarray(w_proj, np.float32))
    res = run_bass_kernel_spmd(nc, in_maps, list(range(N_CORES)), trace=TRACE)
    _CACHE["last_result"] = res

    y = np.zeros((B, T, C), dtype=np.float64)
    for c in range(N_CORES):
        b = c // 4
        y[b] += res.results[c]["y"].astype(np.float64)
    return y.astype(np.float32)


# revision 7
# speedup vs baseline: 1.3361x; 1.0795x over previous
"""Distributed Trainium2 kernel for EnhancedSelfAttention (causal attention
with additive ALiBi |i-j| bias) on 8 NeuronCores.

Math: for queries i and keys j<=i the bias is slope*(i-j), so
softmax_j(S_ij + slope*(i-j)) == softmax_j(S_ij - slope*j) — the slope*i term
is constant per row and cancels. Folding w_j = exp(-slope*j) into V's rows
(plus an appended w column for the denominator) turns the whole softmax into
exp(S) followed by a single PV matmul and a divide. w_j decays so fast that
head h only needs keys with slope_h*j < ~24 (beyond that the dropped weight
is < e^-20 of the total).

Sharding: 8 cores = 2 batches x 4 head groups. Heads are assigned to
(group, slot) sorted by budget so per-slot SPMD budgets (16, 12, 3, 1) are
tight: group g takes heads (15-g, 11-g, 7-g, 3-g). Partials summed on host.

Attention works on S^T tiles ([key, query] layout). Slots are processed in
PAIRS (0,1) and (2,3): slot a's S block goes to columns 0:512 of a shared
[128,1024] PSUM tile via PE row-tile T0 (SBUF partitions 0:63), slot b's to
512:1024 via T8 (64:127) — the two K=64 matmuls run CONCURRENTLY in the
64x128-tiled PE array, and one 1024-wide exp covers both.

Schedule: QKV tranche n (weights x chunk-columns for q-chunk n) is emitted,
then attention q-chunk n for the (0,1) slot pair — so QKV matmuls chase the
input DMAs while early attention's exp (ACT-bound) overlaps later QKV
(PE-bound). PV lags its exp by 2 items; divides are staggered with the den
copy emitted a step early so the PE never stalls on them.

All inputs are pre-swizzled on the host into the exact SBUF layout so every
DMA is a contiguous [128, N] copy at line rate (~420GB/s), split across the
two HWDGE queues (sync + scalar) ordered by first use.
"""

import sys
import types

import numpy as np

import concourse.bass as bass
import concourse.mybir as mybir
import concourse.tile as tile
from concourse import bacc
from concourse.bass_utils import run_bass_kernel_spmd


def _ensure_axon_hooks():
    """concourse's trace path imports antenv.axon_hooks, which this image
    lacks; give it a no-op fallback so BASS_TRACE=1 can't crash the run."""
    try:
        import antenv.axon_hooks  # noqa: F401
    except Exception:
        try:
            import antenv
            mod = types.ModuleType("antenv.axon_hooks")
            mod.get_axon_ntff_profile_hook = lambda: None
            mod.set_axon_ntff_profile_hook = lambda h: None
            sys.modules["antenv.axon_hooks"] = mod
            antenv.axon_hooks = mod
        except Exception:
            pass


_ensure_axon_hooks()

F32 = mybir.dt.float32
F16 = mybir.dt.float16
ExpF = mybir.ActivationFunctionType.Exp

B, T, C = 2, 2048, 1024
NH, D = 16, 64
P = 128
NT = T // P            # 16 t tiles
KC = C // P            # 8 contraction subtiles for qkv/proj
QCH = 4                # q chunks of 512
KBUD = (16, 12, 3, 1)  # per-slot key-block budgets (see docstring)
N_CORES = 8

GROUP_HEADS = [(15 - g, 11 - g, 7 - g, 3 - g) for g in range(4)]

TRACE = False  # test harness sets kernel.TRACE = True for NTFF profiling

_CACHE = {}


def _slopes():
    i = np.arange(1, NH + 1, dtype=np.float64)
    return (1.0 / np.power(2.0, 8.0 * i / NH)).astype(np.float64)


def _build_program():
    nc = bacc.Bacc("TRN2", target_bir_lowering=False, debug=False,
                   num_devices=N_CORES)

    # All host-side arrays are pre-swizzled to [128, free] SBUF layout.
    xt_d = nc.dram_tensor("xt", [P, QCH * KC * 512], F16, kind="ExternalInput").ap()
    wq_d = nc.dram_tensor("wq", [P, KC * 256], F16, kind="ExternalInput").ap()
    wk_d = nc.dram_tensor("wk", [P, KC * 256], F16, kind="ExternalInput").ap()
    wv_d = nc.dram_tensor("wv", [P, KC * 256], F16, kind="ExternalInput").ap()
    wp_d = nc.dram_tensor("wp", [P, 2 * C], F16, kind="ExternalInput").ap()
    wcol_d = nc.dram_tensor("wcol", [P, NT * 4], F32, kind="ExternalInput").ap()
    masks_d = nc.dram_tensor("masks", [P, 4 * 512], F16, kind="ExternalInput").ap()
    y_d = nc.dram_tensor("y", [T, C], F16, kind="ExternalOutput").ap()

    with tile.TileContext(nc) as tc:
        with (
            nc.allow_low_precision(reason="fp16 matmul operands by design"),
            tc.tile_pool(name="const", bufs=1) as const,
            tc.tile_pool(name="psB", bufs=2, space="PSUM") as psB,
            tc.tile_pool(name="psO", bufs=4, space="PSUM") as psO,
            tc.tile_pool(name="pp", bufs=5) as pp,
            tc.tile_pool(name="rr", bufs=3) as rr,
            tc.tile_pool(name="rbp", bufs=3) as rbp,
            tc.tile_pool(name="yp", bufs=4) as yp,
        ):
            # ---- persistent SBUF tiles
            wq_sb = const.tile([P, KC * 256], F16, tag="wq")
            wk_sb = const.tile([P, KC * 256], F16, tag="wk")
            wv_sb = const.tile([P, KC * 256], F16, tag="wv")
            xt_sb = const.tile([P, QCH * KC * 512], F16, tag="xt")  # 32KB/part
            wcol_sb = const.tile([P, NT, 4], F32, tag="wcol")
            masks_sb = const.tile([P, 4 * 512], F16, tag="masks")
            wp_sb = const.tile([P, 2 * C], F16, tag="wp")

            # ---- input DMAs, two HWDGE queues (sync + scalar), ordered by
            # first use.  The first-needed tensors (wq, xt tranche 0) are
            # split across BOTH queues so they land at full aggregate rate.
            NX = KC * 512  # xt columns per q-chunk tranche

            def split_dma(sb, dr, lo, hi):
                mid = (lo + hi) // 2
                nc.sync.dma_start(sb[:, lo:mid], dr[:, lo:mid])
                nc.scalar.dma_start(sb[:, mid:hi], dr[:, mid:hi])

            split_dma(wq_sb, wq_d, 0, KC * 256)
            split_dma(xt_sb, xt_d, 0, NX)
            split_dma(wk_sb, wk_d, 0, KC * 256)
            nc.scalar.dma_start(
                wcol_sb[:], wcol_d[:].rearrange("p (n c) -> p n c", c=4))
            split_dma(wv_sb, wv_d, 0, KC * 256)
            split_dma(xt_sb, xt_d, NX, 2 * NX)
            nc.scalar.dma_start(masks_sb[:], masks_d[:])
            split_dma(xt_sb, xt_d, 2 * NX, 3 * NX)
            split_dma(xt_sb, xt_d, 3 * NX, 4 * NX)
            nc.sync.dma_start(wp_sb[:], wp_d[:])

            ones_sb = const.tile([1, D], F16, tag="ones")
            nc.any.memset(ones_sb[:], 1.0)
            # warm the ACT exp table during the DMA wait
            warm_sb = const.tile([1, D], F16, tag="warm")
            nc.scalar.activation(warm_sb[:], ones_sb[:], ExpF)

            qt_sb = [const.tile([P, T], F16, tag=f"qt{m}", name=f"qt{m}")
                     for m in range(2)]
            kt_sb = [const.tile([P, T], F16, tag=f"kt{m}", name=f"kt{m}")
                     for m in range(2)]
            vv_sb = const.tile([P, NT, 4, 65], F16, tag="vv")
            ot_sb = [const.tile([P, T], F16, tag=f"ot{m}", name=f"ot{m}")
                     for m in range(2)]

            def xt_ap(nch, k, col, width):
                off = nch * NX + k * 512 + col
                return xt_sb[:, off:off + width]

            # ---- emission helpers --------------------------------------
            # QKT group i encodes (w, m, nch): i//8: 0=wq 1=wk;
            # (i%8)//4 = m; i%4 = nch.  Output [d, t] layout.
            def emit_qkt_group(i):
                w_sb, dst = ((wq_sb, qt_sb), (wk_sb, kt_sb))[i // 8]
                m, nch = divmod(i % 8, QCH)
                ps = psB.tile([P, 1024], F32, tag="mm", name="ps_qkt")
                for k in range(KC):
                    nc.tensor.matmul(
                        ps[:, 0:512],
                        w_sb[:, k * 256 + m * P: k * 256 + (m + 1) * P],
                        xt_ap(nch, k, 0, 512),
                        start=(k == 0), stop=(k == KC - 1))
                nc.vector.tensor_copy(
                    dst[m][:, nch * 512:(nch + 1) * 512], ps[:, 0:512])

            def emit_v_group(mt):
                slots = [s for s in range(4) if mt < KBUD[s]]
                nw = slots[-1] * D + D  # used columns are a prefix
                psv = psB.tile([P, 1024], F32, tag="mm", name="ps_v")
                for k in range(KC):
                    nc.tensor.matmul(
                        psv[:, 0:nw],
                        xt_ap(mt // 4, k, (mt % 4) * P, P),
                        wv_sb[:, k * 256:k * 256 + nw],
                        start=(k == 0), stop=(k == KC - 1))
                for s in slots:
                    nc.vector.tensor_scalar_mul(
                        vv_sb[:, mt, s, 0:D], psv[:, s * D:(s + 1) * D],
                        wcol_sb[:, mt, s: s + 1])

            # ---- attention machinery -----------------------------------
            # item = (a, b, qc, g): key block g of S^T for slots a (cols
            # 0:512, PE row-tile T0) and b (512:1024, T8) — concurrent.
            opsums = {}        # (s, qc) -> psum tile
            dens = {}          # (s, qc) -> dh sbuf tile (den row copy)
            pending = []       # up to 2 of (item, pst)
            divq = []          # [(emit_at_step, stage, (s, qc))]
            step = [0]
            ndiv = [0]

            def emit_pv(item, pst):
                a, b, qc, g, has_a, has_b, ka, kb = item
                if has_a:
                    nc.tensor.matmul(
                        opsums[(a, qc)][:], vv_sb[:, g, a, :], pst[:, 0:512],
                        start=(g == 0), stop=(g == ka - 1))
                if has_b:
                    nc.tensor.matmul(
                        opsums[(b, qc)][:], vv_sb[:, g, b, :],
                        pst[:, 512:1024], start=(g == 0), stop=(g == kb - 1))
                out = []
                if has_a and g == ka - 1:
                    out.append((a, qc))
                if has_b and g == kb - 1:
                    out.append((b, qc))
                return out

            def emit_den_copy(s, qc):
                opsum = opsums[(s, qc)]
                dh = rr.tile([1, 512], F16, tag="dh", name="dh")
                if ndiv[0] % 2 == 0:
                    nc.scalar.copy(dh[:], opsum[64:65, :])
                else:
                    nc.vector.tensor_copy(dh[:], opsum[64:65, :])
                ndiv[0] += 1
                dens[(s, qc)] = dh

            def emit_divide(s, qc):
                opsum = opsums.pop((s, qc))
                dh = dens.pop((s, qc))
                ot_t = ot_sb[s // 2]
                base = (s % 2) * D
                rb = psB.tile([P, 1024], F32, tag="mm", name="rb")
                nc.tensor.matmul(rb[0:D, 0:512], ones_sb[:], dh[:],
                                 start=True, stop=True)
                rbs = rbp.tile([D, 512], F32, tag="rbs", name="rbs")
                nc.vector.reciprocal_approx_fast(rbs[:], rb[0:D, 0:512])
                nc.vector.tensor_mul(
                    ot_t[base:base + D, qc * 512:(qc + 1) * 512],
                    opsum[0:64, :], rbs[:])

            def pump_divides():
                while divq and divq[0][0] <= step[0]:
                    _, stage, key = divq.pop(0)
                    (emit_den_copy if stage == 0 else emit_divide)(*key)

            def flush_pending(n_keep):
                while len(pending) > n_keep:
                    done = emit_pv(*pending.pop(0))
                    for j, key in enumerate(done):
                        divq.append((step[0] + 1 + j, 0, key))   # den copy
                        divq.append((step[0] + 2 + j, 1, key))   # divide
                    divq.sort(key=lambda e: (e[0], e[1]))

            def emit_attn_item(item):
                a, b, qc, g, has_a, has_b, ka, kb = item
                pump_divides()
                if g == 0:
                    opsums[(a, qc)] = psO.tile([65, 512], F32, tag="o",
                                               name="opsum_a")
                    opsums[(b, qc)] = psO.tile([65, 512], F32, tag="o",
                                               name="opsum_b")
                m = a // 2
                qt_t, kt_t = qt_sb[m], kt_sb[m]
                sps = psB.tile([P, 1024], F32, tag="mm", name="sps")
                if has_a:
                    nc.tensor.matmul(
                        sps[:, 0:512],
                        kt_t[0:D, g * P:(g + 1) * P],
                        qt_t[0:D, qc * 512:(qc + 1) * 512],
                        start=True, stop=True)
                if has_b:
                    nc.tensor.matmul(
                        sps[:, 512:1024],
                        kt_t[D:2 * D, g * P:(g + 1) * P],
                        qt_t[D:2 * D, qc * 512:(qc + 1) * 512],
                        start=True, stop=True)
                pst = pp.tile([P, 1024], F16, tag="p", name="pst")
                lo, hi = (0, 1024) if (has_a and has_b) else (
                    (0, 512) if has_a else (512, 1024))
                nc.scalar.activation(pst[:, lo:hi], sps[:, lo:hi], ExpF)
                delta = g - 4 * qc
                if 0 <= delta <= 3:  # diagonal block: causal mask per half
                    mk = masks_sb[:, delta * 512:(delta + 1) * 512]
                    if has_a:
                        nc.vector.tensor_mul(pst[:, 0:512], pst[:, 0:512], mk)
                    if has_b:
                        nc.vector.tensor_mul(pst[:, 512:1024],
                                             pst[:, 512:1024], mk)
                flush_pending(2)
                pending.append((item, pst))
                step[0] += 1

            def attn_items(pair, qc):
                a, b = (0, 1) if pair == 0 else (2, 3)
                ka = min(KBUD[a], 4 * qc + 4)
                kb = min(KBUD[b], 4 * qc + 4)
                return [(a, b, qc, g, g < ka, g < kb, ka, kb)
                        for g in range(max(ka, kb))]

            # ---- merged schedule: QKV tranche n, then s01 attention qc=n.
            qkt_by_tranche = [[0, 8, 4, 12], [1, 9, 5], [2, 10, 6], [3, 11, 7]]
            for n in range(4):
                groups = qkt_by_tranche[n]
                vts = list(range(4 * n, 4 * n + 4))
                for j in range(max(len(groups), len(vts))):
                    pump_divides()
                    if j < len(groups):
                        emit_qkt_group(groups[j])
                    if j < len(vts):
                        emit_v_group(vts[j])
                    step[0] += 1
                if n == 0:
                    # den columns for all (t, slot) in one strided copy
                    nc.vector.tensor_copy(vv_sb[:, :, :, 64], wcol_sb[:])
                for item in attn_items(0, n):
                    emit_attn_item(item)
            # s23 attention
            for qc in range(QCH):
                for item in attn_items(1, qc):
                    emit_attn_item(item)
            flush_pending(0)
            # Hoist the first two proj groups' j=0 matmuls (they read only
            # ot_sb[0], complete after the s01 divides) ahead of the divide
            # tail so the PE keeps working through it.
            early_ps = []
            for mt in range(2):
                ps = psB.tile([P, 1024], F32, tag="mm", name="ps_proj_e")
                for nch in range(2):
                    nc.tensor.matmul(
                        ps[:, nch * 512:(nch + 1) * 512],
                        ot_sb[0][:, mt * P:(mt + 1) * P],
                        wp_sb[:, nch * 512:(nch + 1) * 512],
                        start=True, stop=False)
                early_ps.append(ps)
            while divq:
                _, stage, key = divq.pop(0)
                (emit_den_copy if stage == 0 else emit_divide)(*key)

            # ---- phase 3: partial projection y = OT.T @ wp
            for mt in range(NT):
                if mt < 2:
                    ps = early_ps[mt]
                    for nch in range(2):
                        nc.tensor.matmul(
                            ps[:, nch * 512:(nch + 1) * 512],
                            ot_sb[1][:, mt * P:(mt + 1) * P],
                            wp_sb[:, C + nch * 512: C + (nch + 1) * 512],
                            start=False, stop=True)
                else:
                    ps = psB.tile([P, 1024], F32, tag="mm", name="ps_proj")
                    for nch in range(2):
                        for j in range(2):
                            nc.tensor.matmul(
                                ps[:, nch * 512:(nch + 1) * 512],
                                ot_sb[j][:, mt * P:(mt + 1) * P],
                                wp_sb[:, j * C + nch * 512: j * C + (nch + 1) * 512],
                                start=(j == 0), stop=(j == 1))
                yt = yp.tile([P, 1024], F16, tag="y", name="yt")
                if mt % 2 == 0:
                    nc.scalar.copy(yt[:], ps[:])
                else:
                    nc.vector.tensor_copy(yt[:], ps[:])
                eng = nc.sync if mt % 2 == 0 else nc.scalar
                eng.dma_start(y_d[mt * P:(mt + 1) * P, :], yt[:])

    nc.compile()
    return nc


def _host_prep(x, w_qkv, w_proj):
    """Per-core input maps, pre-swizzled to SBUF layout [128, free]."""
    slopes = _slopes()
    scale = 1.0 / np.sqrt(D)
    in_maps = []

    # xt: [P, nch, k, 512] with xt[p, n, k, t'] = x[b][n*512+t', k*128+p]
    xt_by_b = []
    for b in range(B):
        xb = x[b].astype(np.float16)  # [T, C]
        sw = np.ascontiguousarray(
            xb.reshape(QCH, 512, KC, P).transpose(3, 0, 2, 1)
        ).reshape(P, QCH * KC * 512)
        xt_by_b.append(sw)

    # masks: delta in 0..3, [128, 512] each: valid iff r <= c - 128*delta
    rr_ = np.arange(P)[:, None]
    cc = np.arange(512)[None, :]
    masks = np.concatenate(
        [(rr_ <= cc - P * d).astype(np.float16) for d in range(4)], axis=1)

    def swz_w(w):  # [(k p), c] -> [p, (k c)]
        kc = w.shape[1]
        return np.ascontiguousarray(
            w.reshape(KC, P, kc).transpose(1, 0, 2)).reshape(P, KC * kc)

    group_data = []
    for g in range(4):
        H = GROUP_HEADS[g]
        cols = np.concatenate([np.arange(h * D, (h + 1) * D) for h in H])
        wq = swz_w((w_qkv[:, cols] * scale).astype(np.float16))
        wk = swz_w(w_qkv[:, C + cols].astype(np.float16))
        wv = swz_w(w_qkv[:, 2 * C + cols].astype(np.float16))
        wp = np.ascontiguousarray(
            w_proj[cols, :].astype(np.float16).reshape(2, P, C).transpose(1, 0, 2)
        ).reshape(P, 2 * C)
        t = np.arange(T, dtype=np.float64)
        wcol = np.stack(
            [np.exp(-slopes[h] * t) for h in H], axis=1).astype(np.float32)
        wcol = np.ascontiguousarray(
            wcol.reshape(NT, P, 4).transpose(1, 0, 2)).reshape(P, NT * 4)
        group_data.append((wq, wk, wv, wp, wcol))

    for c in range(N_CORES):
        b, g = divmod(c, 4)
        wq, wk, wv, wp, wcol = group_data[g]
        in_maps.append({
            "xt": xt_by_b[b], "wq": wq, "wk": wk, "wv": wv, "wp": wp,
            "wcol": wcol, "masks": masks,
        })
    return in_maps


def kernel(x, w_qkv, w_proj):
    if "nc" not in _CACHE:
        _CACHE["nc"] = _build_program()
    nc = _CACHE["nc"]

    in_maps = _host_prep(np.asarray(x, np.float32), np.asarray(w_qkv, np.float32),
                         np.asarray(w_proj, np.float32))
    res = run_bass_kernel_spmd(nc, in_maps, list(range(N_CORES)), trace=TRACE)
    _CACHE["last_result"] = res

    y = np.zeros((B, T, C), dtype=np.float64)
    for c in range(N_CORES):
        b = c // 4
        y[b] += res.results[c]["y"].astype(np.float64)
    return y.astype(np.float32)
